# revision 28
# baseline (speedup 1.0000x reference)
"""Causal GQA self-attention (B=2, T=2048, C=2048, H=16, KVH=4, HD=128) on 8 TRN2
NeuronCores.

Sharding: one (batch, kv-head-group) pair per core — 2 batches x 4 kv groups = 8
cores. Each core computes, for its batch b and kv group g:
  q = x[b] @ wq[:, g*512:(g+1)*512]      (4 query heads)
  k = x[b] @ wk[:, g*128:(g+1)*128]
  v = x[b] @ wv[:, g*128:(g+1)*128]
  q,k -> RoPE -> RMS-norm; causal softmax(q k^T / sqrt(hd)) @ v
  y_partial = attn_out @ wo[g*512:(g+1)*512, :]
Host sums the 4 partial y's per batch (the O-projection contraction).

Single interleaved instruction stream: projection chunks (A) and attention
t4-blocks (B) are emitted round-robin so the PE never idles on the projection
pipeline's DMA/DVE dependencies and vice versa.  Per-matmul serial overhead
(Ldweights ~60ns, unmodeled by CoreSim) dominates secondary costs, so the
design minimizes matmul count: causal mask is a DVE multiply post-exp (not PE
accumulation), softmax row-sums and the reciprocal broadcast run on the
otherwise-idle GpSimd/Pool engine via partition_all_reduce.

Attention runs in the S^T = K Q^T layout: scores land in [s, t] tiles so the
exp output IS P^T, feeding the PV matmul (V stationary, P^T moving) directly.
k's RMS scale rides the exp() per-partition scale operand for free.

PSUM budget (8 banks): one shared tag "s" ([P,2,512] f32, bufs=3, 6 banks)
rotated by qkv-projection accumulators, S-score tiles, O-projection pieces and
transpose scratch; tag "o" ([P,2,512], bufs=1, 2 banks) for the PV
accumulator.
"""
from collections import deque

import numpy as np
import ml_dtypes

import concourse.bass as bass
import concourse.bass_isa as bass_isa
import concourse.mybir as mybir
import concourse.tile as tile
from concourse import bacc
from concourse.bass_utils import run_bass_kernel_spmd

P = 128          # partitions / head dim
T = 2048         # sequence length
C = 2048         # model dim
NH = 4           # query heads per core (n_rep)
NT = T // P      # 16 t-chunks
NCC = C // P     # 16 contraction chunks
NT4 = 4          # t4 blocks of 512
HD = 128
EPS = 1e-5

f32 = mybir.dt.float32
bf16 = mybir.dt.bfloat16
Exp = mybir.ActivationFunctionType.Exp
Ln = mybir.ActivationFunctionType.Ln
MULT = mybir.AluOpType.mult
ADD = mybir.AluOpType.add


def bcast_mid(ap, n):
    """(P, F) AP -> (P, n, F) with broadcast middle dim."""
    return bass.AP(tensor=ap.tensor, offset=ap.offset,
                   ap=[list(ap.ap[0]), [0, n], list(ap.ap[1])])


# Pin ACT work to the one table set containing BOTH Exp and Ln
# (natural_log_exp_and_others).  The table-load pass assigns each activation
# the first set containing its function, so Exp and Ln otherwise resolve to
# two different sets and the interleaved stream pays a 1.3us table reload on
# every switch.  Feeding the pass a view with Exp/Ln stripped from the other
# sets makes the combined set the unique choice; set ids still index the
# real act_info.json, so the emitted program is unchanged otherwise.
_ORIG_GAT = bacc.get_activation_tables


def _gat_pinned(arch):
    tabs = _ORIG_GAT(arch)
    out = {}
    for name, s in tabs.items():
        if name == "natural_log_exp_and_others":
            out[name] = set(s)
        else:
            out[name] = {f for f in s
                         if str(f).split('.')[-1] not in ("Exp", "Ln")}
    return out


bacc.get_activation_tables = _gat_pinned


def build_program(n_reps=1):
    nc = bacc.Bacc("TRN2", target_bir_lowering=False, debug=False)

    io = {
        "xT": nc.dram_tensor("xT", [C, T], bf16, kind="ExternalInput").ap(),
        "wq": nc.dram_tensor("wq", [C, NH * HD], bf16, kind="ExternalInput").ap(),
        "wkv": nc.dram_tensor("wkv", [C, 2 * HD], bf16, kind="ExternalInput").ap(),
        "wo": nc.dram_tensor("wo", [NH * HD, C], bf16, kind="ExternalInput").ap(),
        "cosx": nc.dram_tensor("cosx", [T, HD // 2], bf16, kind="ExternalInput").ap(),
        "sinx": nc.dram_tensor("sinx", [T, HD // 2], bf16, kind="ExternalInput").ap(),
        "mask01": nc.dram_tensor("mask01", [P, P], bf16, kind="ExternalInput").ap(),
        "identb": nc.dram_tensor("identb", [P, P], bf16, kind="ExternalInput").ap(),
        "y": nc.dram_tensor("y", [T, C], f32, kind="ExternalOutput").ap(),
    }

    with tile.TileContext(nc) as tc:
        with tc.tile_pool(name="const", bufs=1) as const, \
             tc.tile_pool(name="csp", bufs=4) as csp, \
             tc.tile_pool(name="xp", bufs=32) as xp, \
             tc.tile_pool(name="qw", bufs=2) as qw, \
             tc.tile_pool(name="rp", bufs=4) as rp, \
             tc.tile_pool(name="st", bufs=8) as st, \
             tc.tile_pool(name="ptp", bufs=3) as ptp, \
             tc.tile_pool(name="accp", bufs=2) as accp, \
             tc.tile_pool(name="rsp", bufs=2) as rsp, \
             tc.tile_pool(name="yp", bufs=3) as yp, \
             tc.tile_pool(name="ps", bufs=1, space="PSUM") as ps:
            sb = {}
            sb["wq_s"] = const.tile([P, NCC, NH * HD], bf16, name="wq_s")
            sb["wkv_s"] = const.tile([P, NCC, 2 * HD], bf16, name="wkv_s")
            sb["wo_s"] = const.tile([P, NH, C], bf16, name="wo_s")
            wq_r = io["wq"].rearrange("(c p) n -> p c n", p=P)
            wkv_r = io["wkv"].rearrange("(c p) n -> p c n", p=P)
            wo_r = io["wo"].rearrange("(h p) n -> p h n", p=P)
            sb["mask01_s"] = const.tile([P, P], bf16, name="mask01_s")
            sb["identb_s"] = const.tile([P, P], bf16, name="identb_s")
            sb["cos_s"] = const.tile([P, NT, HD // 2], bf16, name="cos_s")
            sb["sin_s"] = const.tile([P, NT, HD // 2], bf16, name="sin_s")
            cos_r = io["cosx"].rearrange("(k p) d -> p k d", p=P)
            sin_r = io["sinx"].rearrange("(k p) d -> p k d", p=P)

            # warm up the ACT function table while the initial DMAs run.
            # The whole program stays on ONE table set
            # (natural_log_exp_and_others: exp+ln+copy) — sqrt is computed
            # as exp(0.5*ln(x)) so no mid-stream table reloads occur.
            warm = const.tile([P, 1], f32, name="warm")
            nc.vector.memset(warm, 1.0)
            warm2 = const.tile([P, 1], f32, name="warm2")
            nc.scalar.activation(warm2, warm, Ln)
            nc.scalar.activation(warm2, warm, Exp)

            sb["qT_all"] = const.tile([P, NH, T], bf16, name="qT_all")
            sb["kT_all"] = const.tile([P, T], bf16, name="kT_all")
            sb["v_all"] = const.tile([P, NT, HD], bf16, name="v_all")
            sb["aoT_s"] = const.tile([P, NH, T], bf16, name="aoT_s")
            sb["rk_all"] = const.tile([P, NT], f32, name="rk_all")

            xts = {}           # (tp, c) -> x tile [P, 512] bf16
            hats = {}          # k -> (qhat, khat) awaiting transpose
            pend = []          # norm/oproj closures (drained via bstream)
            bstream = deque()  # (avail_chunk, est_cycles, closure)

            # ---------------- A: projection chunk ----------------

            def x_load(tp, c):
                t5 = xp.tile([P, 512], bf16, tag="xt", name="xt")
                nc.sync.dma_start(
                    out=t5, in_=io["xT"][c * P:(c + 1) * P,
                                         tp * 512:(tp + 1) * 512])
                xts[(tp, c)] = t5

            def transpose_chunk(j):
                qhat, khat = hats.pop(j)
                ts = slice(j * P, (j + 1) * P)
                pst = ps.tile([P, 2, 512], f32, tag="s", name="pst", bufs=3)
                v = pst.bitcast(bf16)          # [P, 2, 1024]
                for h in range(NH):
                    nc.tensor.matmul(v[:, 0, h * P:(h + 1) * P],
                                     qhat[:, h, :], sb["identb_s"],
                                     is_transpose=True, start=True, stop=True)
                nc.tensor.matmul(v[:, 0, 4 * P:5 * P], khat, sb["identb_s"],
                                 is_transpose=True, start=True, stop=True)
                nc.scalar.copy(sb["qT_all"][:, :, ts], v[:, 0, 0:NH * P])
                nc.scalar.copy(sb["kT_all"][:, ts], v[:, 0, 4 * P:5 * P])

            def emit_A_pair(pair):
                """Fused projection for chunks (2*pair, 2*pair+1) — used for
                the first four chunks, where DMA is the limiter and no
                attention work exists yet: 4 matmuls per (c, weight-load)
                double the PE work per DMA byte.  q pair in one tag-s tile
                (bank per chunk), kv pair packed into a second."""
                qt = ps.tile([P, 2, 512], f32, tag="s", name="qt", bufs=3)
                kvt = ps.tile([P, 2, 512], f32, tag="s", name="kvt", bufs=3)
                for c in range(NCC):
                    if pair == 0:
                        nc.sync.dma_start(out=sb["wq_s"][:, c, :],
                                          in_=wq_r[:, c, :])
                        nc.sync.dma_start(out=sb["wkv_s"][:, c, :],
                                          in_=wkv_r[:, c, :])
                        if c == 1:
                            nc.sync.dma_start(out=sb["mask01_s"],
                                              in_=io["mask01"])
                            nc.sync.dma_start(out=sb["identb_s"],
                                              in_=io["identb"])
                        if c == 2:
                            nc.sync.dma_start(out=sb["cos_s"], in_=cos_r)
                            nc.sync.dma_start(out=sb["sin_s"], in_=sin_r)
                        x_load(0, c)
                    else:
                        if 8 <= c < 12:
                            nc.sync.dma_start(out=sb["wo_s"][:, c - 8, :],
                                              in_=wo_r[:, c - 8, :])
                        if c < 8:
                            x_load(1, 2 * c)
                            x_load(1, 2 * c + 1)
                    xt = xts[(0, c)]
                    for u in range(2):
                        col = (2 * pair + u) * P
                        nc.tensor.matmul(qt[:, u, :], xt[:, col:col + P],
                                         sb["wq_s"][:, c, :],
                                         start=(c == 0), stop=(c == NCC - 1))
                        nc.tensor.matmul(kvt[:, u, 0:2 * HD],
                                         xt[:, col:col + P],
                                         sb["wkv_s"][:, c, :],
                                         start=(c == 0), stop=(c == NCC - 1))
                for u in range(2):
                    finish_chunk(2 * pair + u, qt[:, u, :], kvt[:, u, :])
                if pair == 1:
                    transpose_chunk(0)
                    transpose_chunk(1)

            def emit_A(k):
                tp, col = k // 4, (k % 4) * P
                pqkv = ps.tile([P, 2, 512], f32, tag="s", name="pqkv", bufs=3)
                for c in range(NCC):
                    # x slab prefetch: 4 tiles per chunk during the
                    # preceding 4 chunks
                    if 4 <= k < 12 and c < 4:
                        x_load(k // 4 + 1, 4 * (k % 4) + c)
                    xt = xts[(tp, c)]
                    nc.tensor.matmul(pqkv[:, 0, :], xt[:, col:col + P],
                                     sb["wq_s"][:, c, :],
                                     start=(c == 0), stop=(c == NCC - 1))
                    nc.tensor.matmul(pqkv[:, 1, 0:2 * HD], xt[:, col:col + P],
                                     sb["wkv_s"][:, c, :],
                                     start=(c == 0), stop=(c == NCC - 1))
                # small mid-chunk B drain (<= 2 tag-s bufs are free while
                # pqkv holds the third): bridges the evac+DVE latency below
                drain_b(k, 5000)
                finish_chunk(k, pqkv[:, 0, :], pqkv[:, 1, :])
                # lagged transposes: emitted after this chunk's c-loop so
                # their DVE-chain dependencies have drained by the time the
                # PE reaches them.  emit_A(4) covers chunks 2 and 3 (the
                # second pair), keeping every chunk <= 4*t4+3 transposed
                # before B(t4) steps are appended.
                if k == 4:
                    transpose_chunk(2)
                transpose_chunk(k - 1)

            def finish_chunk(k, q_src, kv_src):
                # evacuate
                q_sb = qw.tile([P, NH, HD], bf16, tag="q_sb", name="q_sb")
                nc.scalar.copy(q_sb.rearrange("p h d -> p (h d)"), q_src)
                k_sb = qw.tile([P, HD], bf16, tag="k_sb", name="k_sb")
                nc.scalar.copy(k_sb, kv_src[:, 0:HD])
                nc.scalar.copy(sb["v_all"][:, k, :], kv_src[:, HD:2 * HD])

                # RMS statistics from pre-rope values (rope is a rotation: it
                # preserves per-row L2 norms)
                msq = st.tile([P, NH + 1], f32, tag="msq", name="msq")
                scr = st.tile([P, HD], bf16, tag="scr", name="scr")
                for h in range(NH):
                    nc.vector.scalar_tensor_tensor(
                        out=scr, in0=q_sb[:, h, :], scalar=1.0,
                        in1=q_sb[:, h, :], op0=MULT, op1=MULT,
                        accum_out=msq[:, h:h + 1])
                nc.vector.scalar_tensor_tensor(
                    out=scr, in0=k_sb, scalar=1.0, in1=k_sb,
                    op0=MULT, op1=MULT, accum_out=msq[:, NH:NH + 1])
                # r = rsqrt(ms + eps) computed as exp(0.5*ln(u2)) to stay on
                # the exp ACT table set (no Sqrt -> no table reloads).  q's
                # 1/sqrt(hd) attention scale folds into Ln's scale arg
                # (sqrt(u2/HD) = exp(0.5*ln(u2*(1/HD)))).  k's rms scale is
                # applied later via exp()'s per-partition scale operand.
                xme = st.tile([P, NH + 1], f32, tag="xme", name="xme")
                nc.vector.tensor_scalar(xme, msq, 1.0 / HD, float(EPS), MULT, ADD)
                u2 = st.tile([P, NH + 1], f32, tag="u", name="u")
                nc.vector.reciprocal(u2, xme)
                lnq = st.tile([P, NH + 1], f32, tag="lnq", name="lnq")
                nc.scalar.activation(lnq[:, 0:NH], u2[:, 0:NH], Ln,
                                     scale=1.0 / HD)
                nc.scalar.activation(lnq[:, NH:NH + 1], u2[:, NH:NH + 1], Ln)
                rr = st.tile([P, NH], f32, tag="rr", name="rr")
                nc.scalar.activation(rr, lnq[:, 0:NH], Exp, scale=0.5)
                nc.scalar.activation(sb["rk_all"][:, k:k + 1],
                                     lnq[:, NH:NH + 1], Exp, scale=0.5)

                # RoPE (halves-split): out1 = x1*c + x2*s ; out2 = x2*c - x1*s
                cos_t = sb["cos_s"][:, k, :]
                sin_t = sb["sin_s"][:, k, :]
                cos_q = bcast_mid(cos_t, NH)
                sin_q = bcast_mid(sin_t, NH)

                qr = qw.tile([P, NH, HD], bf16, tag="qr", name="qr")
                ta = rp.tile([P, NH, HD // 2], bf16, tag="ta", name="ta")
                tb = rp.tile([P, NH, HD // 2], bf16, tag="tb", name="tb")
                tc2 = rp.tile([P, NH, HD // 2], bf16, tag="tc2", name="tc2")
                td2 = rp.tile([P, NH, HD // 2], bf16, tag="td2", name="td2")
                q1, q2 = q_sb[:, :, 0:HD // 2], q_sb[:, :, HD // 2:HD]
                nc.vector.tensor_mul(ta, q1, cos_q)
                nc.vector.tensor_mul(tb, q2, sin_q)
                nc.vector.tensor_add(qr[:, :, 0:HD // 2], ta, tb)
                nc.vector.tensor_mul(tc2, q2, cos_q)
                nc.vector.tensor_mul(td2, q1, sin_q)
                nc.vector.tensor_sub(qr[:, :, HD // 2:HD], tc2, td2)

                # k rope straight to bf16 (no rms mult: rk rides exp's scale)
                khat = qw.tile([P, HD], bf16, tag="khat", name="khat", bufs=5)
                ka = rp.tile([P, HD // 2], bf16, tag="ka", name="ka")
                kb = rp.tile([P, HD // 2], bf16, tag="kb", name="kb")
                k1, k2 = k_sb[:, 0:HD // 2], k_sb[:, HD // 2:HD]
                nc.vector.tensor_mul(ka, k1, cos_t)
                nc.vector.tensor_mul(kb, k2, sin_t)
                nc.vector.tensor_add(khat[:, 0:HD // 2], ka, kb)
                nc.vector.tensor_mul(ka, k2, cos_t)
                nc.vector.tensor_mul(kb, k1, sin_t)
                nc.vector.tensor_sub(khat[:, HD // 2:HD], ka, kb)

                # apply RMS scale (q also gets the 1/sqrt(hd) attention scale)
                qhat = qw.tile([P, NH, HD], bf16, tag="qhat", name="qhat",
                               bufs=5)
                for h in range(NH):
                    nc.vector.tensor_scalar(qhat[:, h, :], qr[:, h, :],
                                            rr[:, h:h + 1], None, MULT)
                hats[k] = (qhat, khat)

            # ---------------- B: attention t4-block steps ----------------

            def oproj_piece(m, cc):
                yt = ps.tile([P, 2, 512], f32, tag="s", name="yt", bufs=3)
                ys = yt[:, 0, :]
                for h in range(NH):
                    nc.tensor.matmul(
                        ys, sb["aoT_s"][:, h, m * P:(m + 1) * P],
                        sb["wo_s"][:, h, cc * 512:(cc + 1) * 512],
                        start=(h == 0), stop=(h == NH - 1))
                y_sb = yp.tile([P, 512], f32, tag="y_sb", name="y_sb")
                if cc % 2 == 0:
                    nc.scalar.copy(y_sb, ys)
                else:
                    nc.vector.tensor_copy(y_sb, ys)
                nc.sync.dma_start(
                    out=io["y"][m * P:(m + 1) * P, cc * 512:(cc + 1) * 512],
                    in_=y_sb)

            def make_b_steps(t4):
                """Closures for t4's attention: per pr, the sj stream (S ->
                exp -> mask -> acc -> lagged PV), then Pool rowsum + evac;
                finally the reciprocal + deferred norm/oproj pieces."""
                avail = 4 * t4 + 4 if t4 < 3 else 16
                n_sj = 4 * t4 + 4
                t_lo = t4 * 512
                steps = []
                sd = [{}, {}]    # per-pr state

                def sj_step(pr, sj):
                    d = sd[pr]
                    m0 = max(0, sj - 4 * t4)
                    w0 = m0 * P
                    diag = sj >= 4 * t4
                    ps2 = ps.tile([P, 2, 512], f32, tag="s", name="ps2", bufs=3)
                    pt2 = ptp.tile([P, 2, 512], bf16, tag="pt", name="pt2")
                    for i in range(2):
                        h = pr * 2 + i
                        nc.tensor.matmul(
                            ps2[:, i, w0:512],
                            sb["kT_all"][:, sj * P:(sj + 1) * P],
                            sb["qT_all"][:, h, t_lo + w0:t_lo + 512],
                            start=True, stop=True)
                    nc.scalar.activation(pt2[:, :, w0:512], ps2[:, :, w0:512],
                                         Exp, scale=sb["rk_all"][:, sj:sj + 1])
                    if diag:  # causal mask: zero strict upper block on DVE
                        nc.vector.tensor_mul(
                            pt2[:, :, w0:w0 + P], pt2[:, :, w0:w0 + P],
                            bcast_mid(sb["mask01_s"], 2))
                    if sj == 0:
                        acc2 = accp.tile([P, 2, 512], bf16, tag="acc",
                                         name="acc2")
                        nc.vector.tensor_copy(acc2, pt2)
                        d["acc2"] = acc2
                        d["pso2"] = ps.tile([P, 2, 512], f32, tag="o",
                                            name="pso2", bufs=1)
                    else:
                        nc.vector.tensor_add(d["acc2"][:, :, w0:512],
                                             d["acc2"][:, :, w0:512],
                                             pt2[:, :, w0:512])
                    if "prev" in d:
                        pj, pw0, ppt = d["prev"]
                        for i in range(2):
                            nc.tensor.matmul(d["pso2"][:, i, pw0:512],
                                             sb["v_all"][:, pj, :],
                                             ppt[:, i, pw0:512],
                                             start=(pj == 0), stop=False,
                                             skip_group_check=True)
                    d["prev"] = (sj, w0, pt2)

                def pr_finish(pr):
                    d = sd[pr]
                    pj, pw0, ppt = d["prev"]
                    for i in range(2):
                        nc.tensor.matmul(d["pso2"][:, i, pw0:512],
                                         sb["v_all"][:, pj, :],
                                         ppt[:, i, pw0:512],
                                         start=(pj == 0), stop=True,
                                         skip_group_check=True)
                    # softmax denominators: partition all-reduce on Pool
                    # (emitted first so it overlaps the DVE evac below),
                    # reciprocal immediately so this pr's norm pieces can
                    # drain during the other pr's sj loop
                    rs = rsp.tile([P, 2, 512], bf16, tag="rs", name="rs")
                    nc.gpsimd.partition_all_reduce(rs, d["acc2"], 128,
                                                   bass_isa.ReduceOp.add)
                    # early (unnormalized) evac of the attention output
                    # (ACT: overlaps the Pool reduce and the DVE reciprocal)
                    nc.scalar.copy(
                        sb["aoT_s"][:, pr * 2:(pr + 1) * 2, t_lo:t_lo + 512],
                        d["pso2"])
                    rcp = rsp.tile([P, 2, 512], bf16, tag=f"rcp{pr}",
                                   name="rcp")
                    with nc.allow_low_precision(reason="softmax denom "
                                                "reciprocal; bf16 is ample"):
                        nc.vector.reciprocal(rcp, rs)
                    d["rcp"] = rcp

                def norm_piece(h):
                    rcp = sd[h // 2]["rcp"]
                    nc.vector.tensor_mul(
                        sb["aoT_s"][:, h, t_lo:t_lo + 512],
                        sb["aoT_s"][:, h, t_lo:t_lo + 512], rcp[:, h % 2, :])

                for pr in range(2):
                    for sj in range(n_sj):
                        w0 = max(0, sj - 4 * t4) * P
                        cyc = 2 * (512 - w0) + 1024 + 580
                        steps.append((avail, cyc,
                                      lambda pr=pr, sj=sj: sj_step(pr, sj)))
                    steps.append((avail, 1024 + 290,
                                  lambda pr=pr: pr_finish(pr)))
                    steps.append((avail, 0,
                                  lambda pr=pr: norm_piece(2 * pr)))
                    steps.append((avail, 0,
                                  lambda pr=pr: norm_piece(2 * pr + 1)))
                oavail = avail + 1 if t4 < 3 else avail
                for u in range(4):
                    for cc in range(4):
                        steps.append((oavail, 2048 + 580,
                                      lambda m=t4 * 4 + u, cc=cc:
                                      oproj_piece(m, cc)))
                return steps

            # ---------------- interleaved emission ----------------

            QUOTA = [0] * 4 + [14000] * 4 + [19000] * 4 + [26000] * 4

            def drain_b(k, budget):
                while bstream and bstream[0][0] <= k and budget > 0:
                    _, cyc, fn = bstream.popleft()
                    fn()
                    budget -= cyc

            for rep in range(n_reps):
                xts.clear()
                hats.clear()
                emit_A_pair(0)
                emit_A_pair(1)
                for k in range(4, NT):
                    emit_A(k)
                    if k % 4 == 0:
                        bstream.extend(make_b_steps((k - 4) // 4))
                    drain_b(k, QUOTA[k])
                transpose_chunk(NT - 1)
                bstream.extend(make_b_steps(3))
                drain_b(NT, float("inf"))

    nc.compile()
    return nc


_PROG = None


def _get_prog():
    global _PROG
    if _PROG is None:
        _PROG = build_program()
    return _PROG


def make_in_maps(x, cos, sin, wq, wk, wv, wo):
    """Shard full inputs into 8 per-core input dicts."""
    cosf = np.ascontiguousarray(cos.reshape(T, HD // 2)).astype(ml_dtypes.bfloat16)
    sinf = np.ascontiguousarray(sin.reshape(T, HD // 2)).astype(ml_dtypes.bfloat16)
    ii, jj = np.indices((P, P))
    mask01 = np.where(jj >= ii, 1.0, 0.0).astype(ml_dtypes.bfloat16)
    identb = np.eye(P, dtype=np.float32).astype(ml_dtypes.bfloat16)

    in_maps = []
    for g in range(8):
        b, kv = divmod(g, 4)
        sl4 = slice(kv * NH * HD, (kv + 1) * NH * HD)   # 512 wide
        sl1 = slice(kv * HD, (kv + 1) * HD)             # 128 wide
        in_maps.append({
            "xT": np.ascontiguousarray(x[b].T).astype(ml_dtypes.bfloat16),
            "wq": np.ascontiguousarray(wq[:, sl4]).astype(ml_dtypes.bfloat16),
            "wkv": np.ascontiguousarray(
                np.concatenate([wk[:, sl1], wv[:, sl1]], axis=1)).astype(
                    ml_dtypes.bfloat16),
            "wo": np.ascontiguousarray(wo[sl4, :]).astype(ml_dtypes.bfloat16),
            "cosx": cosf, "sinx": sinf,
            "mask01": mask01, "identb": identb,
        })
    return in_maps


def kernel(x, cos, sin, wq, wk, wv, wo, window_size=0):
    x = np.asarray(x); cos = np.asarray(cos); sin = np.asarray(sin)
    wq = np.asarray(wq); wk = np.asarray(wk); wv = np.asarray(wv)
    wo = np.asarray(wo)
    prog = _get_prog()
    in_maps = make_in_maps(x, cos, sin, wq, wk, wv, wo)
    res = run_bass_kernel_spmd(prog, in_maps, core_ids=list(range(8)))
    outs = [r["y"] for r in res.results]
    yfull = np.empty((2, T, C), dtype=np.float32)
    for b in range(2):
        yfull[b] = outs[4 * b] + outs[4 * b + 1] + outs[4 * b + 2] + outs[4 * b + 3]
    return yfull
